# revision 17
# baseline (speedup 1.0000x reference)
"""Trainium2 Bass kernel for nn_Attention (B=16, N=1024, C=768, H=12).

Strategy: pure data parallelism - batch 16 sharded 2-per-core across 8
NeuronCores, weights replicated, no collectives.

v3b changes vs v2 (400us):
  - b_proj applied by the DVE during the proj PSUM->SBUF copy
    (broadcast bias tile) instead of ones-row matmuls: ~7us less PE.
  - weight DMA order: q cols + k pair 0 first, then the v half needed
    by the first pairs, then the rest of k, then v half 1, wproj, and
    the batch-1 x cast last.  v2 loaded all of v after q/k pair 0,
    which starved the first pair's AV chain (et pool filled, exp
    stalled ~15us at startup).
  - all 8 v(0,*,0) units run in the startup block instead of as pair
    (0,0) fillers, so AV never waits on a v tile.
  - batch-1 yT is transposed in 4 chunks at their earliest dependency
    points ([0,512) after pair 2, [512,768) after pair 4, [768,896)
    after pair 5's first token half, [896,1024) at the end), and projs
    are scheduled per-chunk, so only proj(tb=7) trails the last pair.
"""

import numpy as np

import concourse.bass as bass
import concourse.tile as tile
from concourse import bacc, mybir
from concourse.bass_utils import run_bass_kernel_spmd
from concourse.masks import make_identity

F32 = mybir.dt.float32
BF16 = mybir.dt.bfloat16
AF = mybir.ActivationFunctionType

P = 128
B_LOC = 2
N = 1024
C = 768
H = 12
D = 64
CB = C // P
NB = N // P
SCALE = D ** -0.5
EXP_BIAS = -4.0  # constant shift inside exp; cancels in softmax
VW = 72          # v tile col stride (65 used: 64 data + ones)


def _build():
    nc = bacc.Bacc(None, target_bir_lowering=False)

    x_h = nc.declare_dram_parameter("x", [B_LOC, N, C], F32, isOutput=False)
    wqkv_h = nc.declare_dram_parameter("w_qkv", [C, 3 * C], F32, isOutput=False)
    wproj_h = nc.declare_dram_parameter("w_proj", [C, C], F32, isOutput=False)
    bproj_h = nc.declare_dram_parameter("b_proj", [C], F32, isOutput=False)
    out_h = nc.declare_dram_parameter("out", [B_LOC, N, C], F32, isOutput=True)

    scratch = nc.dram_tensor("scratch", [B_LOC, H * N, D], BF16)
    xbf = nc.dram_tensor("xbf", [N, C], BF16)  # batch 1 only

    with tile.TileContext(nc) as tc:
        from contextlib import ExitStack

        with ExitStack() as ctx:
            ep = ctx.enter_context

            const = ep(tc.tile_pool(name="const", bufs=1))
            xstg = ep(tc.tile_pool(name="xstg", bufs=4))
            wpool = ep(tc.tile_pool(name="weights", bufs=1))
            xTp = ep(tc.tile_pool(name="xT", bufs=2))
            qkTp = ep(tc.tile_pool(name="qkT", bufs=2))
            vp = ep(tc.tile_pool(name="v", bufs=2 * 2 * NB))
            epool = ep(tc.tile_pool(name="etiles", bufs=6))
            otp = ep(tc.tile_pool(name="ot", bufs=4))
            otokp = ep(tc.tile_pool(name="otok", bufs=2))
            osbp = ep(tc.tile_pool(name="osb", bufs=2))
            rcp = ep(tc.tile_pool(name="rc", bufs=4))
            yTp = ep(tc.tile_pool(name="yT", bufs=2))
            zp = ep(tc.tile_pool(name="z", bufs=2))

            psum_gp = ep(tc.tile_pool(name="psum_gp", bufs=2, space="PSUM"))
            psum_st = ep(tc.tile_pool(name="psum_st", bufs=2, space="PSUM"))
            psum_ot = ep(tc.tile_pool(name="psum_ot", bufs=2, space="PSUM"))

            ident_f32 = const.tile([P, P], F32)
            make_identity(nc, ident_f32[:])
            exp_bias = const.tile([P, 1], F32)
            nc.vector.memset(exp_bias[:], EXP_BIAS)

            # preload the exp table set during startup (first real exp
            # otherwise pays the ~2.7us ACT_TABLE_LOAD inline)
            exp_dummy = const.tile([1, 8], F32)
            nc.vector.memset(exp_dummy[:], 0.0)
            nc.scalar.activation(exp_dummy[:], exp_dummy[:], AF.Exp)

            # bias broadcast across all 128 partitions (for the DVE add
            # during the proj PSUM->SBUF copy)
            bias_rep = const.tile([P, C], F32, name="bias_rep")
            nc.sync.dma_start(
                bias_rep[:],
                bproj_h[:].rearrange("(o c) -> o c", o=1).to_broadcast([P, C]))

            # ---- weights to SBUF as bf16 (gpsimd DMAs cast f32->bf16) ----
            wqkv_bf = []
            for cb in range(CB):
                wt = wpool.tile([P, 3 * C], BF16, tag=f"wqkv{cb}",
                                name=f"wqkv{cb}")
                # q (all pairs) + k pair 0
                nc.gpsimd.dma_start(
                    wt[:, 0:896], wqkv_h[cb * P:(cb + 1) * P, 0:896])
                wqkv_bf.append(wt)
            for cb in range(CB):   # v heads 0-5 (first three pairs)
                nc.gpsimd.dma_start(
                    wqkv_bf[cb][:, 2 * C:2 * C + 384],
                    wqkv_h[cb * P:(cb + 1) * P, 2 * C:2 * C + 384])
            for cb in range(CB):   # k pairs 1-5
                nc.gpsimd.dma_start(
                    wqkv_bf[cb][:, 896:2 * C],
                    wqkv_h[cb * P:(cb + 1) * P, 896:2 * C])
            for cb in range(CB):   # v heads 6-11
                nc.gpsimd.dma_start(
                    wqkv_bf[cb][:, 2 * C + 384:3 * C],
                    wqkv_h[cb * P:(cb + 1) * P, 2 * C + 384:3 * C])
            wproj_bf = []
            for cb in range(CB):
                wt = wpool.tile([P, C], BF16, tag=f"wproj{cb}",
                                name=f"wproj{cb}")
                nc.gpsimd.dma_start(wt[:], wproj_h[cb * P:(cb + 1) * P, :])
                wproj_bf.append(wt)

            state = {}

            def get_xT(b):
                if ("xT", b) not in state:
                    state[("xT", b)] = [
                        xTp.tile([P, N], BF16, tag=f"xT{cb}", name=f"xT{cb}")
                        for cb in range(CB)]
                return state[("xT", b)]

            def get_yT(b):
                if ("yT", b) not in state:
                    state[("yT", b)] = [
                        yTp.tile([P, N], BF16, tag=f"yT{cb2}",
                                 name=f"yT{cb2}") for cb2 in range(CB)]
                return state[("yT", b)]

            def emit_x0_tbs(tbs):
                # batch 0 startup: stage f32 rows, PE-transpose (warms PE)
                xT = get_xT(0)
                for tb in tbs:
                    xs = xstg.tile([P, C], F32, tag="xs", name="xs")
                    nc.sync.dma_start(xs[:], x_h[0, tb * P:(tb + 1) * P, :])
                    for cb in range(CB):
                        pt = psum_gp.tile([P, P], F32, tag="gp", name="ptx")
                        nc.tensor.transpose(
                            pt[:], xs[:, cb * P:(cb + 1) * P], ident_f32[:])
                        nc.vector.tensor_copy(
                            out=xT[cb][:, tb * P:(tb + 1) * P], in_=pt[:])

            def emit_x1_path():
                # batch 1: cast to bf16 in DRAM, XBAR-transpose to SBUF
                xT = get_xT(1)
                for half in range(2):
                    nc.gpsimd.dma_start(
                        xbf[half * 512:(half + 1) * 512, :],
                        x_h[1, half * 512:(half + 1) * 512, :])
                for cb in range(CB):
                    nc.sync.dma_start(
                        xT[cb][:], xbf[:, cb * P:(cb + 1) * P],
                        transpose=True)

            def emit_qk_unit(b, fb, th):
                xT = get_xT(b)
                if ("qkT", b) not in state:
                    state[("qkT", b)] = [
                        qkTp.tile([P, N], BF16, tag=f"qkT{fb2}",
                                  name=f"qkT{fb2}") for fb2 in range(12)]
                qkT = state[("qkT", b)]
                ps = psum_gp.tile([P, 512], F32, tag="gp", name="psqk")
                for cb in range(CB):
                    nc.tensor.matmul(
                        ps[:],
                        wqkv_bf[cb][:, fb * P:(fb + 1) * P],
                        xT[cb][:, th * 512:(th + 1) * 512],
                        start=(cb == 0), stop=(cb == CB - 1))
                nc.vector.tensor_copy(
                    out=qkT[fb][:, th * 512:(th + 1) * 512], in_=ps[:])

            def emit_v_unit(b, tb, vh):
                xT = get_xT(b)
                ps = psum_gp.tile([P, 384], F32, tag="gp", name="psv")
                f0 = 2 * C + vh * 384
                for cb in range(CB):
                    nc.tensor.matmul(
                        ps[:],
                        xT[cb][:, tb * P:(tb + 1) * P],
                        wqkv_bf[cb][:, f0:f0 + 384],
                        start=(cb == 0), stop=(cb == CB - 1))
                vt = vp.tile([P, 6, VW], BF16, tag="vt", name="vt")
                nc.vector.tensor_copy(
                    out=vt[:, :, 0:64],
                    in_=ps[:].rearrange("p (h d) -> p h d", d=64))
                nc.vector.memset(vt[:, :, 64:65], 1.0)
                state[("v", b, tb, vh)] = vt

            def emit_head_pair(b, j, fillers=(), split_tail=False):
                fillers = list(fillers)
                hA, hB = 2 * j, 2 * j + 1
                qkT = state[("qkT", b)]
                qA, qB = qkT[j][0:64, :], qkT[j][64:128, :]
                kA, kB = qkT[6 + j][0:64, :], qkT[6 + j][64:128, :]
                ot_A = otp.tile([80, N], BF16, tag="ot_sb", name="otA")
                ot_B = otp.tile([80, N], BF16, tag="ot_sb", name="otB")
                nc.vector.memset(ot_A[64:80, :], 0.0)
                nc.vector.memset(ot_B[64:80, :], 0.0)

                def norm_half(ot_sb, h, nh, eng=None):
                    # transpose [80, 512] -> [128, 4, 80]: token
                    # nh*512 + e*128 + p lands at [p, e, :]
                    otok = otokp.tile([P, 4, 80], BF16, tag="otok",
                                      name="otok")
                    (eng or nc.sync).dma_start(
                        otok[:], ot_sb[:, nh * 512:(nh + 1) * 512],
                        transpose=True)
                    rc = rcp.tile([P, 4, 1], F32, tag="rc", name="rc")
                    nc.vector.reciprocal(rc[:, :, 0], otok[:, :, 64])
                    osb = osbp.tile([P, 4, D], BF16, tag="osb", name="osb")
                    nc.vector.tensor_mul(
                        osb[:], otok[:, :, 0:64],
                        rc[:].to_broadcast([P, 4, D]))
                    dst = scratch[b, h * N + nh * 512:
                                  h * N + (nh + 1) * 512, :]
                    nc.gpsimd.dma_start(
                        dst.rearrange("(e p) d -> p e d", p=P), osb[:])

                def norm_full(ot_sb, h):
                    otok = otokp.tile([P, NB, 80], BF16, tag="otokf",
                                      name="otokf")
                    nc.sync.dma_start(otok[:], ot_sb[:], transpose=True)
                    rc = rcp.tile([P, NB, 1], F32, tag="rcf", name="rcf")
                    nc.vector.reciprocal(rc[:, :, 0], otok[:, :, 64])
                    osb = osbp.tile([P, NB, D], BF16, tag="osbf",
                                    name="osbf")
                    nc.vector.tensor_mul(
                        osb[:], otok[:, :, 0:64],
                        rc[:].to_broadcast([P, NB, D]))
                    dst = scratch[b, h * N:(h + 1) * N, :]
                    nc.gpsimd.dma_start(
                        dst.rearrange("(e p) d -> p e d", p=P), osb[:])

                for nh in range(2):
                    potA = psum_ot.tile([65, 512], F32, tag="pot",
                                        name="potA")
                    potB = psum_ot.tile([65, 512], F32, tag="pot",
                                        name="potB")
                    pend = []
                    for mb in range(NB):
                        vA = state[("v", b, mb, hA // 6)][:, hA % 6, 0:65]
                        vB = state[("v", b, mb, hB // 6)][:, hB % 6, 0:65]
                        pst = psum_st.tile([P, N], F32, tag="st", name="pst")
                        nc.tensor.matmul(
                            pst[:, 0:512],
                            kA[:, mb * P:(mb + 1) * P],
                            qA[:, nh * 512:(nh + 1) * 512],
                            start=True, stop=True)
                        nc.tensor.matmul(
                            pst[:, 512:1024],
                            kB[:, mb * P:(mb + 1) * P],
                            qB[:, nh * 512:(nh + 1) * 512],
                            start=True, stop=True)
                        et = epool.tile([P, N], BF16, tag="et", name="et")
                        nc.scalar.activation(
                            et[:], pst[:], AF.Exp,
                            bias=exp_bias[:], scale=SCALE)
                        pend.append((mb, et, vA, vB))
                        # AV lags S/exp by 2 m-blocks: when AV(k) issues,
                        # S(k+2) has already waited on exp(k)'s pst slot,
                        # so the PE never stalls on the ACT exp.
                        if len(pend) > 2:
                            pmb, pet, pvA, pvB = pend.pop(0)
                            nc.tensor.matmul(
                                potA[:], pvA, pet[:, 0:512],
                                start=(pmb == 0), stop=False)
                            nc.tensor.matmul(
                                potB[:], pvB, pet[:, 512:1024],
                                start=(pmb == 0), stop=False)
                            if fillers:
                                fillers.pop(0)()
                    for pmb, pet, pvA, pvB in pend:
                        nc.tensor.matmul(
                            potA[:], pvA, pet[:, 0:512],
                            start=(pmb == 0), stop=(pmb == NB - 1))
                        nc.tensor.matmul(
                            potB[:], pvB, pet[:, 512:1024],
                            start=(pmb == 0), stop=(pmb == NB - 1))
                    nc.vector.tensor_copy(
                        out=ot_A[0:65, nh * 512:(nh + 1) * 512], in_=potA[:])
                    nc.vector.tensor_copy(
                        out=ot_B[0:65, nh * 512:(nh + 1) * 512], in_=potB[:])
                    if split_tail:
                        # nh1: the scalar queue has no exps left by then,
                        # so run one transpose there in parallel
                        norm_half(ot_A, hA, nh)
                        norm_half(ot_B, hB, nh,
                                  eng=nc.scalar if nh == 1 else None)
                if not split_tail:
                    norm_full(ot_A, hA)
                    norm_full(ot_B, hB)
                for f in fillers:
                    f()

            def emit_yT(b, n0, n1, split_engines=False):
                # y rows [n0, n1) only touch scratch rows 12n'+ch, so a
                # row range depends only on the head pairs covering
                # [12*n0, 12*n1) - lets proj start before all heads done.
                yT = get_yT(b)
                y_view = scratch[b].rearrange("(n ch) d -> n (ch d)", ch=H)
                for cb in range(CB):
                    eng = nc.scalar if (split_engines and cb % 2) else nc.sync
                    eng.dma_start(
                        yT[cb][:, n0:n1],
                        y_view[n0:n1, cb * P:(cb + 1) * P],
                        transpose=True)

            def emit_proj_tb(b, tb):
                yT = get_yT(b)
                z_sb = zp.tile([P, C], F32, tag="z_sb", name="z_sb")
                for zh, zw in ((0, 512), (512, 256)):
                    pz = psum_gp.tile([P, zw], F32, tag="gp", name="pz")
                    for cb in range(CB):
                        nc.tensor.matmul(
                            pz[:], yT[cb][:, tb * P:(tb + 1) * P],
                            wproj_bf[cb][:, zh:zh + zw],
                            start=(cb == 0), stop=(cb == CB - 1))
                    nc.vector.tensor_add(
                        z_sb[:, zh:zh + zw], pz[:], bias_rep[:, zh:zh + zw])
                    nc.gpsimd.dma_start(
                        out_h[b, tb * P:(tb + 1) * P, zh:zh + zw],
                        z_sb[:, zh:zh + zw])

            # ---------- emission schedule ----------
            def F(fn, *a):
                return lambda: fn(*a)

            emit_x0_tbs(range(0, 4))
            emit_qk_unit(0, 0, 0)
            emit_qk_unit(0, 6, 0)
            emit_x0_tbs(range(4, 8))
            emit_qk_unit(0, 0, 1)
            emit_qk_unit(0, 6, 1)
            for tb in range(3):
                emit_v_unit(0, tb, 0)
            emit_x1_path()

            qkf = lambda b, fb: [F(emit_qk_unit, b, fb, 0),
                                 F(emit_qk_unit, b, fb, 1)]
            vf = lambda b, tb, vh: F(emit_v_unit, b, tb, vh)
            pj = lambda b, tb: F(emit_proj_tb, b, tb)
            nop = lambda: None
            # 6 filler slots pop per nh half (mb 2..7); leftovers run at
            # the end of the pair.  A filler that READS data must be
            # emitted after its writers (Tile gives no RAW edge to a
            # reader emitted first).
            pair_fill = {
                (0, 0): [vf(0, tb, 0) for tb in range(3, 8)]
                    + qkf(0, 1) + qkf(0, 7),
                (0, 1): qkf(0, 2) + qkf(0, 8) + [
                    vf(0, tb, 1) for tb in range(0, 4)],
                (0, 2): qkf(0, 3) + qkf(0, 9) + [
                    vf(0, tb, 1) for tb in range(4, 8)],
                (0, 3): qkf(0, 4) + qkf(0, 10) + qkf(1, 0) + qkf(1, 6),
                (0, 4): qkf(0, 5) + qkf(0, 11) + qkf(1, 1) + qkf(1, 7),
                (0, 5): qkf(1, 2) + qkf(1, 8) + [
                    vf(1, tb, 0) for tb in range(0, 4)],
                (1, 0): [vf(1, tb, 0) for tb in range(4, 8)]
                    + qkf(1, 3) + qkf(1, 9),
                (1, 1): qkf(1, 4) + qkf(1, 10) + [
                    vf(1, tb, 1) for tb in range(0, 4)]
                    + [pj(0, 0), pj(0, 1)],
                (1, 2): qkf(1, 5) + qkf(1, 11) + [
                    vf(1, tb, 1) for tb in range(4, 8)]
                    + [pj(0, 2), pj(0, 3)],
                (1, 3): [pj(0, 4), pj(0, 5), pj(0, 6), pj(0, 7),
                         pj(1, 0), pj(1, 1)],
                (1, 4): [pj(1, 2), pj(1, 3), pj(1, 4)],
                (1, 5): [pj(1, 5), nop, nop, nop, nop, nop,
                         F(emit_yT, 1, 768, 896), pj(1, 6)],
            }

            # yT chunk dependency map (y row n' needs scratch rows
            # 12n'+ch, i.e. heads 12n'/1024 ..): [0,512) <- pairs 0-2,
            # [512,640) <- pair 3, [640,768) <- pairs 3-4,
            # [768,896) <- pair 4 + pair 5 half 0, [896,1024) <- pair 5.
            for b in range(2):
                for j in range(6):
                    emit_head_pair(b, j, pair_fill[(b, j)],
                                   split_tail=(b, j) == (1, 5))
                    if (b, j) == (0, 2):
                        emit_yT(0, 0, 512)
                    if (b, j) == (0, 3):
                        emit_yT(0, 512, 640)
                    if (b, j) == (0, 4):
                        emit_yT(0, 640, 768)
                    if (b, j) == (0, 5):
                        emit_yT(0, 768, 1024)
                    if (b, j) == (1, 2):
                        emit_yT(1, 0, 512)
                    if (b, j) == (1, 3):
                        emit_yT(1, 512, 640)
                    if (b, j) == (1, 4):
                        emit_yT(1, 640, 768)
            emit_yT(1, 896, 1024, split_engines=True)
            emit_proj_tb(1, 7)

    nc.compile()
    return nc


_NC_CACHE = {}


def _get_nc():
    if "nc" not in _NC_CACHE:
        _NC_CACHE["nc"] = _build()
    return _NC_CACHE["nc"]


def kernel(x, w_qkv, w_proj, b_proj, _trace=False):
    nc = _get_nc()
    n_cores = 8
    x = np.ascontiguousarray(x, dtype=np.float32)
    w_qkv = np.ascontiguousarray(w_qkv, dtype=np.float32)
    w_proj = np.ascontiguousarray(w_proj, dtype=np.float32)
    b_proj = np.ascontiguousarray(b_proj, dtype=np.float32)
    in_maps = [
        {
            "x": x[i * B_LOC:(i + 1) * B_LOC],
            "w_qkv": w_qkv,
            "w_proj": w_proj,
            "b_proj": b_proj,
        }
        for i in range(n_cores)
    ]
    res = run_bass_kernel_spmd(
        nc, in_maps, core_ids=list(range(n_cores)), trace=_trace)
    out = np.concatenate([res.results[i]["out"] for i in range(n_cores)], axis=0)
    if _trace:
        return out, res
    return out


# revision 19
# speedup vs baseline: 1.0466x; 1.0466x over previous
"""Trainium2 Bass kernel for nn_Attention (B=16, N=1024, C=768, H=12).

Strategy: pure data parallelism - batch 16 sharded 2-per-core across 8
NeuronCores, weights replicated, no collectives.

v3b changes vs v2 (400us):
  - b_proj applied by the DVE during the proj PSUM->SBUF copy
    (broadcast bias tile) instead of ones-row matmuls: ~7us less PE.
  - weight DMA order: q cols + k pair 0 first, then the v half needed
    by the first pairs, then the rest of k, then v half 1, wproj, and
    the batch-1 x cast last.  v2 loaded all of v after q/k pair 0,
    which starved the first pair's AV chain (et pool filled, exp
    stalled ~15us at startup).
  - all 8 v(0,*,0) units run in the startup block instead of as pair
    (0,0) fillers, so AV never waits on a v tile.
  - batch-1 yT is transposed in 4 chunks at their earliest dependency
    points ([0,512) after pair 2, [512,768) after pair 4, [768,896)
    after pair 5's first token half, [896,1024) at the end), and projs
    are scheduled per-chunk, so only proj(tb=7) trails the last pair.
"""

import numpy as np

import concourse.bass as bass
import concourse.tile as tile
from concourse import bacc, mybir
from concourse.bass_utils import run_bass_kernel_spmd
from concourse.masks import make_identity

F32 = mybir.dt.float32
BF16 = mybir.dt.bfloat16
AF = mybir.ActivationFunctionType

P = 128
B_LOC = 2
N = 1024
C = 768
H = 12
D = 64
CB = C // P
NB = N // P
SCALE = D ** -0.5
EXP_BIAS = -4.0  # constant shift inside exp; cancels in softmax
VW = 72          # v tile col stride (65 used: 64 data + ones)


def _build():
    nc = bacc.Bacc(None, target_bir_lowering=False)

    x_h = nc.declare_dram_parameter("x", [B_LOC, N, C], F32, isOutput=False)
    wqkv_h = nc.declare_dram_parameter("w_qkv", [C, 3 * C], F32, isOutput=False)
    wproj_h = nc.declare_dram_parameter("w_proj", [C, C], F32, isOutput=False)
    bproj_h = nc.declare_dram_parameter("b_proj", [C], F32, isOutput=False)
    out_h = nc.declare_dram_parameter("out", [B_LOC, N, C], F32, isOutput=True)

    scratch = nc.dram_tensor("scratch", [B_LOC, H * N, D], BF16)
    xbf = nc.dram_tensor("xbf", [N, C], BF16)  # batch 1 only

    with tile.TileContext(nc) as tc:
        from contextlib import ExitStack

        with ExitStack() as ctx:
            ep = ctx.enter_context

            const = ep(tc.tile_pool(name="const", bufs=1))
            xstg = ep(tc.tile_pool(name="xstg", bufs=4))
            wpool = ep(tc.tile_pool(name="weights", bufs=1))
            xTp = ep(tc.tile_pool(name="xT", bufs=2))
            qkTp = ep(tc.tile_pool(name="qkT", bufs=2))
            vp = ep(tc.tile_pool(name="v", bufs=2 * 2 * NB))
            epool = ep(tc.tile_pool(name="etiles", bufs=6))
            otp = ep(tc.tile_pool(name="ot", bufs=4))
            otokp = ep(tc.tile_pool(name="otok", bufs=2))
            osbp = ep(tc.tile_pool(name="osb", bufs=2))
            rcp = ep(tc.tile_pool(name="rc", bufs=4))
            yTp = ep(tc.tile_pool(name="yT", bufs=2))
            zp = ep(tc.tile_pool(name="z", bufs=2))

            psum_gp = ep(tc.tile_pool(name="psum_gp", bufs=2, space="PSUM"))
            psum_st = ep(tc.tile_pool(name="psum_st", bufs=2, space="PSUM"))
            psum_ot = ep(tc.tile_pool(name="psum_ot", bufs=2, space="PSUM"))

            ident_f32 = const.tile([P, P], F32)
            make_identity(nc, ident_f32[:])
            exp_bias = const.tile([P, 1], F32)
            nc.vector.memset(exp_bias[:], EXP_BIAS)

            # preload the exp table set during startup (first real exp
            # otherwise pays the ~2.7us ACT_TABLE_LOAD inline)
            exp_dummy = const.tile([1, 8], F32)
            nc.vector.memset(exp_dummy[:], 0.0)
            nc.scalar.activation(exp_dummy[:], exp_dummy[:], AF.Exp)

            # bias broadcast across all 128 partitions (for the DVE add
            # during the proj PSUM->SBUF copy).  On the scalar queue: the
            # sync queue must start the x staging immediately (this DMA
            # first on sync costs ~10us of PE idle at startup), and the
            # scalar queue is free until the first exp.
            bias_rep = const.tile([P, C], F32, name="bias_rep")
            nc.scalar.dma_start(
                bias_rep[:],
                bproj_h[:].rearrange("(o c) -> o c", o=1).to_broadcast([P, C]))

            # ---- weights to SBUF as bf16 (gpsimd DMAs cast f32->bf16) ----
            wqkv_bf = []
            for cb in range(CB):
                wt = wpool.tile([P, 3 * C], BF16, tag=f"wqkv{cb}",
                                name=f"wqkv{cb}")
                # q (all pairs) + k pair 0
                nc.gpsimd.dma_start(
                    wt[:, 0:896], wqkv_h[cb * P:(cb + 1) * P, 0:896])
                wqkv_bf.append(wt)
            for cb in range(CB):   # v heads 0-5 (first three pairs)
                nc.gpsimd.dma_start(
                    wqkv_bf[cb][:, 2 * C:2 * C + 384],
                    wqkv_h[cb * P:(cb + 1) * P, 2 * C:2 * C + 384])
            for cb in range(CB):   # k pairs 1-5
                nc.gpsimd.dma_start(
                    wqkv_bf[cb][:, 896:2 * C],
                    wqkv_h[cb * P:(cb + 1) * P, 896:2 * C])
            for cb in range(CB):   # v heads 6-11
                nc.gpsimd.dma_start(
                    wqkv_bf[cb][:, 2 * C + 384:3 * C],
                    wqkv_h[cb * P:(cb + 1) * P, 2 * C + 384:3 * C])
            wproj_bf = []
            for cb in range(CB):
                wt = wpool.tile([P, C], BF16, tag=f"wproj{cb}",
                                name=f"wproj{cb}")
                nc.gpsimd.dma_start(wt[:], wproj_h[cb * P:(cb + 1) * P, :])
                wproj_bf.append(wt)

            state = {}

            def get_xT(b):
                if ("xT", b) not in state:
                    state[("xT", b)] = [
                        xTp.tile([P, N], BF16, tag=f"xT{cb}", name=f"xT{cb}")
                        for cb in range(CB)]
                return state[("xT", b)]

            def get_yT(b):
                if ("yT", b) not in state:
                    state[("yT", b)] = [
                        yTp.tile([P, N], BF16, tag=f"yT{cb2}",
                                 name=f"yT{cb2}") for cb2 in range(CB)]
                return state[("yT", b)]

            def emit_x0_tbs(tbs):
                # batch 0 startup: stage f32 rows, PE-transpose (warms PE)
                xT = get_xT(0)
                for tb in tbs:
                    xs = xstg.tile([P, C], F32, tag="xs", name="xs")
                    nc.sync.dma_start(xs[:], x_h[0, tb * P:(tb + 1) * P, :])
                    for cb in range(CB):
                        pt = psum_gp.tile([P, P], F32, tag="gp", name="ptx")
                        nc.tensor.transpose(
                            pt[:], xs[:, cb * P:(cb + 1) * P], ident_f32[:])
                        nc.vector.tensor_copy(
                            out=xT[cb][:, tb * P:(tb + 1) * P], in_=pt[:])

            def emit_x1_path():
                # batch 1: cast to bf16 in DRAM, XBAR-transpose to SBUF
                xT = get_xT(1)
                for half in range(2):
                    nc.gpsimd.dma_start(
                        xbf[half * 512:(half + 1) * 512, :],
                        x_h[1, half * 512:(half + 1) * 512, :])
                for cb in range(CB):
                    nc.sync.dma_start(
                        xT[cb][:], xbf[:, cb * P:(cb + 1) * P],
                        transpose=True)

            def emit_qk_unit(b, fb, th):
                xT = get_xT(b)
                if ("qkT", b) not in state:
                    state[("qkT", b)] = [
                        qkTp.tile([P, N], BF16, tag=f"qkT{fb2}",
                                  name=f"qkT{fb2}") for fb2 in range(12)]
                qkT = state[("qkT", b)]
                ps = psum_gp.tile([P, 512], F32, tag="gp", name="psqk")
                for cb in range(CB):
                    nc.tensor.matmul(
                        ps[:],
                        wqkv_bf[cb][:, fb * P:(fb + 1) * P],
                        xT[cb][:, th * 512:(th + 1) * 512],
                        start=(cb == 0), stop=(cb == CB - 1))
                nc.vector.tensor_copy(
                    out=qkT[fb][:, th * 512:(th + 1) * 512], in_=ps[:])

            def emit_v_unit(b, tb, vh):
                xT = get_xT(b)
                ps = psum_gp.tile([P, 384], F32, tag="gp", name="psv")
                f0 = 2 * C + vh * 384
                for cb in range(CB):
                    nc.tensor.matmul(
                        ps[:],
                        xT[cb][:, tb * P:(tb + 1) * P],
                        wqkv_bf[cb][:, f0:f0 + 384],
                        start=(cb == 0), stop=(cb == CB - 1))
                vt = vp.tile([P, 6, VW], BF16, tag="vt", name="vt")
                nc.vector.tensor_copy(
                    out=vt[:, :, 0:64],
                    in_=ps[:].rearrange("p (h d) -> p h d", d=64))
                nc.vector.memset(vt[:, :, 64:65], 1.0)
                state[("v", b, tb, vh)] = vt

            def emit_head_pair(b, j, fillers=(), split_tail=False):
                fillers = list(fillers)
                hA, hB = 2 * j, 2 * j + 1
                qkT = state[("qkT", b)]
                qA, qB = qkT[j][0:64, :], qkT[j][64:128, :]
                kA, kB = qkT[6 + j][0:64, :], qkT[6 + j][64:128, :]
                ot_A = otp.tile([80, N], BF16, tag="ot_sb", name="otA")
                ot_B = otp.tile([80, N], BF16, tag="ot_sb", name="otB")
                nc.vector.memset(ot_A[64:80, :], 0.0)
                nc.vector.memset(ot_B[64:80, :], 0.0)

                def norm_half(ot_sb, h, nh, eng=None):
                    # transpose [80, 512] -> [128, 4, 80]: token
                    # nh*512 + e*128 + p lands at [p, e, :]
                    otok = otokp.tile([P, 4, 80], BF16, tag="otok",
                                      name="otok")
                    (eng or nc.sync).dma_start(
                        otok[:], ot_sb[:, nh * 512:(nh + 1) * 512],
                        transpose=True)
                    rc = rcp.tile([P, 4, 1], F32, tag="rc", name="rc")
                    nc.vector.reciprocal(rc[:, :, 0], otok[:, :, 64])
                    osb = osbp.tile([P, 4, D], BF16, tag="osb", name="osb")
                    nc.vector.tensor_mul(
                        osb[:], otok[:, :, 0:64],
                        rc[:].to_broadcast([P, 4, D]))
                    dst = scratch[b, h * N + nh * 512:
                                  h * N + (nh + 1) * 512, :]
                    nc.gpsimd.dma_start(
                        dst.rearrange("(e p) d -> p e d", p=P), osb[:])

                def norm_full(ot_sb, h):
                    otok = otokp.tile([P, NB, 80], BF16, tag="otokf",
                                      name="otokf")
                    nc.sync.dma_start(otok[:], ot_sb[:], transpose=True)
                    rc = rcp.tile([P, NB, 1], F32, tag="rcf", name="rcf")
                    nc.vector.reciprocal(rc[:, :, 0], otok[:, :, 64])
                    osb = osbp.tile([P, NB, D], BF16, tag="osbf",
                                    name="osbf")
                    nc.vector.tensor_mul(
                        osb[:], otok[:, :, 0:64],
                        rc[:].to_broadcast([P, NB, D]))
                    dst = scratch[b, h * N:(h + 1) * N, :]
                    nc.gpsimd.dma_start(
                        dst.rearrange("(e p) d -> p e d", p=P), osb[:])

                for nh in range(2):
                    potA = psum_ot.tile([65, 512], F32, tag="pot",
                                        name="potA")
                    potB = psum_ot.tile([65, 512], F32, tag="pot",
                                        name="potB")
                    pend = []
                    for mb in range(NB):
                        vA = state[("v", b, mb, hA // 6)][:, hA % 6, 0:65]
                        vB = state[("v", b, mb, hB // 6)][:, hB % 6, 0:65]
                        pst = psum_st.tile([P, N], F32, tag="st", name="pst")
                        nc.tensor.matmul(
                            pst[:, 0:512],
                            kA[:, mb * P:(mb + 1) * P],
                            qA[:, nh * 512:(nh + 1) * 512],
                            start=True, stop=True)
                        nc.tensor.matmul(
                            pst[:, 512:1024],
                            kB[:, mb * P:(mb + 1) * P],
                            qB[:, nh * 512:(nh + 1) * 512],
                            start=True, stop=True)
                        et = epool.tile([P, N], BF16, tag="et", name="et")
                        nc.scalar.activation(
                            et[:], pst[:], AF.Exp,
                            bias=exp_bias[:], scale=SCALE)
                        pend.append((mb, et, vA, vB))
                        # AV lags S/exp by 2 m-blocks: when AV(k) issues,
                        # S(k+2) has already waited on exp(k)'s pst slot,
                        # so the PE never stalls on the ACT exp.
                        if len(pend) > 2:
                            pmb, pet, pvA, pvB = pend.pop(0)
                            nc.tensor.matmul(
                                potA[:], pvA, pet[:, 0:512],
                                start=(pmb == 0), stop=False)
                            nc.tensor.matmul(
                                potB[:], pvB, pet[:, 512:1024],
                                start=(pmb == 0), stop=False)
                            if fillers:
                                fillers.pop(0)()
                    for pmb, pet, pvA, pvB in pend:
                        nc.tensor.matmul(
                            potA[:], pvA, pet[:, 0:512],
                            start=(pmb == 0), stop=(pmb == NB - 1))
                        nc.tensor.matmul(
                            potB[:], pvB, pet[:, 512:1024],
                            start=(pmb == 0), stop=(pmb == NB - 1))
                    nc.vector.tensor_copy(
                        out=ot_A[0:65, nh * 512:(nh + 1) * 512], in_=potA[:])
                    nc.vector.tensor_copy(
                        out=ot_B[0:65, nh * 512:(nh + 1) * 512], in_=potB[:])
                    if split_tail:
                        # nh1: the scalar queue has no exps left by then,
                        # so run one transpose there in parallel
                        norm_half(ot_A, hA, nh)
                        norm_half(ot_B, hB, nh,
                                  eng=nc.scalar if nh == 1 else None)
                if not split_tail:
                    norm_full(ot_A, hA)
                    norm_full(ot_B, hB)
                for f in fillers:
                    f()

            def emit_yT(b, n0, n1, split_engines=False):
                # y rows [n0, n1) only touch scratch rows 12n'+ch, so a
                # row range depends only on the head pairs covering
                # [12*n0, 12*n1) - lets proj start before all heads done.
                yT = get_yT(b)
                y_view = scratch[b].rearrange("(n ch) d -> n (ch d)", ch=H)
                for cb in range(CB):
                    eng = nc.scalar if (split_engines and cb % 2) else nc.sync
                    eng.dma_start(
                        yT[cb][:, n0:n1],
                        y_view[n0:n1, cb * P:(cb + 1) * P],
                        transpose=True)

            def emit_proj_tb(b, tb):
                yT = get_yT(b)
                z_sb = zp.tile([P, C], F32, tag="z_sb", name="z_sb")
                for zh, zw in ((0, 512), (512, 256)):
                    pz = psum_gp.tile([P, zw], F32, tag="gp", name="pz")
                    for cb in range(CB):
                        nc.tensor.matmul(
                            pz[:], yT[cb][:, tb * P:(tb + 1) * P],
                            wproj_bf[cb][:, zh:zh + zw],
                            start=(cb == 0), stop=(cb == CB - 1))
                    nc.vector.tensor_add(
                        z_sb[:, zh:zh + zw], pz[:], bias_rep[:, zh:zh + zw])
                    nc.gpsimd.dma_start(
                        out_h[b, tb * P:(tb + 1) * P, zh:zh + zw],
                        z_sb[:, zh:zh + zw])

            # ---------- emission schedule ----------
            def F(fn, *a):
                return lambda: fn(*a)

            emit_x0_tbs(range(0, 4))
            emit_qk_unit(0, 0, 0)
            emit_qk_unit(0, 6, 0)
            emit_x0_tbs(range(4, 8))
            emit_qk_unit(0, 0, 1)
            emit_qk_unit(0, 6, 1)
            for tb in range(3):
                emit_v_unit(0, tb, 0)
            emit_x1_path()

            qkf = lambda b, fb: [F(emit_qk_unit, b, fb, 0),
                                 F(emit_qk_unit, b, fb, 1)]
            vf = lambda b, tb, vh: F(emit_v_unit, b, tb, vh)
            pj = lambda b, tb: F(emit_proj_tb, b, tb)
            nop = lambda: None
            # 6 filler slots pop per nh half (mb 2..7); leftovers run at
            # the end of the pair.  A filler that READS data must be
            # emitted after its writers (Tile gives no RAW edge to a
            # reader emitted first).
            pair_fill = {
                (0, 0): [vf(0, tb, 0) for tb in range(3, 8)]
                    + qkf(0, 1) + qkf(0, 7),
                (0, 1): qkf(0, 2) + qkf(0, 8) + [
                    vf(0, tb, 1) for tb in range(0, 4)],
                (0, 2): qkf(0, 3) + qkf(0, 9) + [
                    vf(0, tb, 1) for tb in range(4, 8)],
                (0, 3): qkf(0, 4) + qkf(0, 10) + qkf(1, 0) + qkf(1, 6),
                (0, 4): qkf(0, 5) + qkf(0, 11) + qkf(1, 1) + qkf(1, 7),
                (0, 5): qkf(1, 2) + qkf(1, 8) + [
                    vf(1, tb, 0) for tb in range(0, 4)],
                (1, 0): [vf(1, tb, 0) for tb in range(4, 8)]
                    + qkf(1, 3) + qkf(1, 9),
                (1, 1): qkf(1, 4) + qkf(1, 10) + [
                    vf(1, tb, 1) for tb in range(0, 4)]
                    + [pj(0, 0), pj(0, 1)],
                (1, 2): qkf(1, 5) + qkf(1, 11) + [
                    vf(1, tb, 1) for tb in range(4, 8)]
                    + [pj(1, 0)],
                (1, 3): [pj(0, 2), pj(0, 3), pj(0, 4), pj(0, 5),
                         pj(1, 1)],
                (1, 4): [pj(0, 6), pj(0, 7), pj(1, 2), pj(1, 3)],
                (1, 5): [pj(1, 4), pj(1, 5), nop, nop, nop, nop,
                         F(emit_yT, 1, 848, 896), pj(1, 6)],
            }

            # yT chunks: y row n' needs scratch rows 12n'+ch, i.e. head
            # pair k covers rows up to ~(2k+2)*1024/12.  Each chunk is
            # emitted right after its last head pair, so the transposes
            # run (on the sync queue) one full pair before any proj
            # filler reads them - a proj filler whose yT transposes
            # haven't run yet head-of-line-blocks the whole PE queue.
            chunk_after = {2: (0, 512), 3: (512, 672), 4: (672, 848)}
            fine_after = {0: (0, 160), 1: (160, 336), 2: (336, 512),
                          3: (512, 672), 4: (672, 848)}
            for b in range(2):
                for j in range(6):
                    emit_head_pair(b, j, pair_fill[(b, j)],
                                   split_tail=(b, j) == (1, 5))
                    chunks = fine_after if b == 1 else chunk_after
                    if j in chunks:
                        emit_yT(b, *chunks[j])
                    if (b, j) == (0, 5):
                        emit_yT(0, 848, 1024)
            emit_yT(1, 896, 1024, split_engines=True)
            emit_proj_tb(1, 7)

    nc.compile()
    return nc


_NC_CACHE = {}


def _get_nc():
    if "nc" not in _NC_CACHE:
        _NC_CACHE["nc"] = _build()
    return _NC_CACHE["nc"]


def kernel(x, w_qkv, w_proj, b_proj, _trace=False):
    nc = _get_nc()
    n_cores = 8
    x = np.ascontiguousarray(x, dtype=np.float32)
    w_qkv = np.ascontiguousarray(w_qkv, dtype=np.float32)
    w_proj = np.ascontiguousarray(w_proj, dtype=np.float32)
    b_proj = np.ascontiguousarray(b_proj, dtype=np.float32)
    in_maps = [
        {
            "x": x[i * B_LOC:(i + 1) * B_LOC],
            "w_qkv": w_qkv,
            "w_proj": w_proj,
            "b_proj": b_proj,
        }
        for i in range(n_cores)
    ]
    res = run_bass_kernel_spmd(
        nc, in_maps, core_ids=list(range(n_cores)), trace=_trace)
    out = np.concatenate([res.results[i]["out"] for i in range(n_cores)], axis=0)
    if _trace:
        return out, res
    return out
